# revision 10
# baseline (speedup 1.0000x reference)
"""Trainium2 Bass kernel: 12-head MHA (B=2, S=4096, D=768) sharded over 8 cores.

Sharding: batch x head-group. Core c handles batch b=c//4 and heads
3*(c%4) .. 3*(c%4)+2 (Megatron-style column split of Wq/Wk/Wv, row split
of Wo; the Wo all-reduce is the host-side gather/sum).

Per-core device algorithm ("R2b"):
  - Projections on PE with biases folded in via an augmented ones-row in
    the contraction dim.  Q^T,K^T produced dk-major [64, S]; V produced
    seq-major [S, 192].
  - PATH-B (per head): scores^T tiles [128k, q] on PE, exp on ScalarE
    (unnormalized, bf16), A^T @ V accumulated on PE -> O_un^T [64, S].
  - PATH-A (per head): scores tiles [128q, k] on PE, exp on ScalarE with
    accum_out giving softmax row-sums for free, DVE normalize, DMA out
    full f32 attention weights (the dominant 1.6 GB output).
  - Final: per-head O_un^T @ Wo^T partial products, scaled by the
    per-(head,row) reciprocal row-sum during PSUM evacuation, summed on
    DVE, DMA out f32 partial output; host sums the 4 partials per batch
    and adds bo.
"""

import os
import sys

import numpy as np

_REPO = "/opt/trn_rl_repo"
if _REPO not in sys.path:
    sys.path.insert(0, _REPO)

import ml_dtypes  # noqa: E402

BF16NP = ml_dtypes.bfloat16

B, S, D, H, DK = 2, 4096, 768, 12, 64
HPC = 3            # heads per core
HD = HPC * DK      # 192 head dims per core
NCORES = 8
SCALE = 1.0 / 8.0  # 1/sqrt(DK)

_CACHE = {}


def _build(s=S):
    """Build and compile the per-core Bass/Tile kernel (SPMD; same NEFF on
    every core)."""
    import concourse.bacc as bacc
    import concourse.mybir as mybir
    import concourse.tile as tile

    f32 = mybir.dt.float32
    bf16 = mybir.dt.bfloat16

    nc = bacc.Bacc(
        "TRN2",
        target_bir_lowering=False,
        debug=False,
        enable_asserts=False,
        num_devices=NCORES,
    )

    t = {}
    for n in ("xqT", "xkT", "xvT"):
        t[n] = nc.dram_tensor(n, [D + 1, s], bf16, kind="ExternalInput").ap()
    for n in ("wqT", "wkT", "wvT"):
        t[n] = nc.dram_tensor(n, [D + 1, HD], bf16, kind="ExternalInput").ap()
    t["woT"] = nc.dram_tensor("woT", [HD, D], bf16, kind="ExternalInput").ap()
    t["attn"] = nc.dram_tensor("attn", [HPC, s, s], f32, kind="ExternalOutput").ap()
    t["outp"] = nc.dram_tensor("outp", [s, D], f32, kind="ExternalOutput").ap()

    with tile.TileContext(nc) as tc:
        _emit(tc, nc, t, s, mybir)

    nc.compile()
    return nc


def _emit(tc, nc, t, s, mybir):
    from contextlib import ExitStack

    f32 = mybir.dt.float32
    bf16 = mybir.dt.bfloat16
    Exp = mybir.ActivationFunctionType.Exp

    NQ5 = s // 512    # 512-wide tiles
    NQT = s // 128    # 128-wide tiles
    NKC = s // 128    # contraction chunks of 128 over seq
    BW = min(1024, s)         # PATH-B q-tile width (psum-limited)
    NB = s // BW
    AW = min(2048, s)         # PATH-A exp width over k
    NA = s // AW
    NCH = D // 128            # 6 weight/x chunks of 128 (+1 ones row)

    # Head layout: heads 0,1 are "packed" into 128-partition tiles
    # (h0 rows 0:64, h1 rows 64:128) so their K=64 / M=64 matmuls can
    # run concurrently in disjoint halves of the PE array; head 2 is
    # standalone in 64-row tiles.
    P0, P1 = slice(0, 64), slice(64, 128)

    with ExitStack() as ctx:
        persist = ctx.enter_context(tc.tile_pool(name="persist", bufs=1))

        qtP = persist.tile([128, s], bf16, name="qtP", tag="qtP")
        ktP = persist.tile([128, s], bf16, name="ktP", tag="ktP")
        qt2 = persist.tile([DK, s], bf16, name="qt2", tag="qt2")
        kt2 = persist.tile([DK, s], bf16, name="kt2", tag="kt2")
        vsb = persist.tile([128, NKC, HD], bf16, name="vsb", tag="vsb")
        otP = persist.tile([128, s], bf16, name="otP", tag="otP")
        ot2 = persist.tile([DK, s], bf16, name="ot2", tag="ot2")
        woP = persist.tile([128, D], bf16, name="woP", tag="woP")
        wo2 = persist.tile([DK, D], bf16, name="wo2", tag="wo2")
        recip = persist.tile([128, HPC * NQT], f32, name="recip", tag="recip")

        nc.sync.dma_start(woP[:, :], t["woT"][0:128, :])
        nc.sync.dma_start(wo2[:, :], t["woT"][128:HD, :])

        # ---------------- projections ----------------
        with (
            tc.tile_pool(name="wch", bufs=1) as wpool,
            tc.tile_pool(name="xch", bufs=1) as xpool,
            tc.tile_pool(name="pps", bufs=2, space="PSUM") as pps,
        ):
            wch = {}
            for n in ("wqT", "wkT", "wvT"):
                chunks = []
                for c in range(NCH):
                    w = wpool.tile([128, HD], bf16, name=f"{n}{c}", tag=f"{n}{c}")
                    nc.sync.dma_start(w[:, :], t[n][c * 128:(c + 1) * 128, :])
                    chunks.append(w)
                w = wpool.tile([1, HD], bf16, name=f"{n}6", tag=f"{n}6")
                nc.sync.dma_start(w[:, :], t[n][D:D + 1, :])
                chunks.append(w)
                wch[n] = chunks

            def load_x(n):
                xs = []
                for c in range(NCH):
                    x = xpool.tile([128, s], bf16, name=f"x{c}", tag=f"x{c}")
                    nc.sync.dma_start(x[:, :], t[n][c * 128:(c + 1) * 128, :])
                    xs.append(x)
                x = xpool.tile([1, s], bf16, name="x6", tag="x6")
                nc.sync.dma_start(x[:, :], t[n][D:D + 1, :])
                xs.append(x)
                return xs

            # Q^T and K^T: dk-major; heads 0,1 in one 128-row tile, head 2 alone
            for xn, wn, outP, out2 in (("xqT", "wqT", qtP, qt2),
                                       ("xkT", "wkT", ktP, kt2)):
                xs = load_x(xn)
                for q5 in range(NQ5):
                    qs = slice(q5 * 512, q5 * 512 + 512)
                    psP = pps.tile([128, 512], f32, name="psP", tag="psP")
                    ps2 = pps.tile([DK, 512], f32, name="ps2", tag="ps2")
                    for c in range(NCH + 1):
                        nc.tensor.matmul(psP[:, :], wch[wn][c][:, 0:128],
                                         xs[c][:, qs],
                                         start=(c == 0), stop=(c == NCH))
                        nc.tensor.matmul(ps2[:, :], wch[wn][c][:, 128:HD],
                                         xs[c][:, qs],
                                         start=(c == 0), stop=(c == NCH))
                    nc.vector.tensor_copy(outP[:, qs], psP[:, :])
                    nc.vector.tensor_copy(out2[:, qs], ps2[:, :])

            # V: seq-major [s, 192]
            xs = load_x("xvT")
            for qt in range(NQT):
                qs = slice(qt * 128, qt * 128 + 128)
                ps = pps.tile([128, HD], f32, name="psv", tag="psv")
                for c in range(NCH + 1):
                    nc.tensor.matmul(ps[:, :], xs[c][:, qs], wch["wvT"][c][:, :],
                                     start=(c == 0), stop=(c == NCH))
                nc.vector.tensor_copy(vsb[:, qt, :], ps[:, :])

        # ---------------- attention: per-head, A/B interleaved ----------------
        # For each head, PATH-A (q-major scores -> normalized attn out) and
        # PATH-B (k-major scores -> A@V) are emitted interleaved so the two
        # exp-paced pipelines keep both PE and ACT dense.
        def ab_phase(h, qA, kA, vc0, otdst, oslice):
            with (
                tc.tile_pool(name=f"aps{h}", bufs=1, space="PSUM") as aps,
                tc.tile_pool(name=f"bps{h}", bufs=1, space="PSUM") as bps,
                tc.tile_pool(name=f"au{h}", bufs=2) as aup,
                tc.tile_pool(name=f"ark{h}", bufs=2) as arkp,
                tc.tile_pool(name=f"but{h}", bufs=3) as butp,
            ):
                avp = None
                for qt in range(NQT):
                    qs = slice(qt * 128, qt * 128 + 128)
                    # --- PATH-A unit ---
                    u = aup.tile([128, s], f32, name=f"u{h}", tag=f"u{h}")
                    rk = arkp.tile([128, max(NA, 2)], f32,
                                   name=f"rk{h}", tag=f"rk{h}")
                    for half in range(NA):
                        sp = aps.tile([128, AW], f32, name=f"sp{h}",
                                      tag=f"sp{h}", bufs=1)
                        for sub in range(AW // 512):
                            ks = slice(half * AW + sub * 512,
                                       half * AW + sub * 512 + 512)
                            nc.tensor.matmul(sp[:, sub * 512:(sub + 1) * 512],
                                             qA[:, qs], kA[:, ks],
                                             start=True, stop=True)
                        nc.scalar.activation(
                            u[:, half * AW:(half + 1) * AW], sp[:, :],
                            Exp, scale=SCALE, accum_out=rk[:, half:half + 1])
                    rsum = arkp.tile([128, 1], f32, name=f"rs{h}", tag=f"rs{h}")
                    if NA == 2:
                        nc.vector.tensor_add(rsum[:, :], rk[:, 0:1], rk[:, 1:2])
                    else:
                        nc.vector.tensor_copy(rsum[:, :], rk[:, 0:1])
                    rc = recip[:, h * NQT + qt:h * NQT + qt + 1]
                    nc.vector.reciprocal(rc, rsum[:, :])
                    nc.vector.tensor_scalar_mul(u[:, :], u[:, :], rc)
                    nc.sync.dma_start(t["attn"][h, qs, :], u[:, :])
                    # --- PATH-B units: NKC*NQ5/NQT per A unit ---
                    nbu = (NKC * NQ5) // NQT
                    for j in range(nbu):
                        idx = qt * nbu + j
                        q5 = idx // NKC
                        kc = idx % NKC
                        qbs = slice(q5 * 512, q5 * 512 + 512)
                        st = bps.tile([128, 512], f32, name=f"st{h}",
                                      tag=f"st{h}", bufs=2)
                        nc.tensor.matmul(st[:, :],
                                         kA[:, kc * 128:(kc + 1) * 128],
                                         qA[:, qbs], start=True, stop=True)
                        ut = butp.tile([128, 512], bf16, name=f"ut{h}",
                                       tag=f"ut{h}")
                        nc.scalar.activation(ut[:, :], st[:, :], Exp, scale=SCALE)
                        if kc == 0:
                            avp = bps.tile([DK, 512], f32, name=f"av{h}",
                                           tag=f"av{h}", bufs=2)
                        nc.tensor.matmul(avp[:, :], vsb[:, kc, vc0:vc0 + DK],
                                         ut[:, :],
                                         start=(kc == 0), stop=(kc == NKC - 1))
                        if kc == NKC - 1:
                            nc.vector.tensor_copy(otdst[oslice, qbs], avp[:, :])

        ab_phase(0, qtP[P0, :], ktP[P0, :], 0, otP, P0)
        ab_phase(1, qtP[P1, :], ktP[P1, :], DK, otP, P1)
        ab_phase(2, qt2, kt2, 2 * DK, ot2, slice(0, DK))

        # ---------------- output projection (partial) ----------------
        with (
            tc.tile_pool(name="fps", bufs=1, space="PSUM") as fps,
            tc.tile_pool(name="facc", bufs=2) as faccp,
            tc.tile_pool(name="ftmp", bufs=2) as ftmpp,
        ):
            for qt in range(NQT):
                qs = slice(qt * 128, qt * 128 + 128)
                acc = faccp.tile([128, D], f32, name="acc", tag="acc")
                srcs = [(0, otP[P0, :], woP[P0, :]),
                        (1, otP[P1, :], woP[P1, :]),
                        (2, ot2, wo2)]
                pfs = []
                for h, oA, wA in srcs:
                    pf = fps.tile([128, D], f32, name=f"pf{h}", tag=f"pf{h}")
                    nc.tensor.matmul(pf[:, 0:512], oA[:, qs], wA[:, 0:512],
                                     start=True, stop=True)
                    nc.tensor.matmul(pf[:, 512:D], oA[:, qs], wA[:, 512:D],
                                     start=True, stop=True)
                    pfs.append((h, pf))
                for h, pf in pfs:
                    rc = recip[:, h * NQT + qt:h * NQT + qt + 1]
                    if h == 0:
                        nc.vector.tensor_scalar_mul(acc[:, :], pf[:, :], rc)
                    else:
                        tmp = ftmpp.tile([128, D], f32, name=f"tmp{h}",
                                         tag=f"tmp{h}")
                        nc.vector.tensor_scalar_mul(tmp[:, :], pf[:, :], rc)
                        nc.vector.tensor_add(acc[:, :], acc[:, :], tmp[:, :])
                nc.sync.dma_start(t["outp"][qs, :], acc[:, :])


def make_in_maps(inputs, s=S):
    """Shard + lay out the full inputs for the 8 cores."""
    q = np.asarray(inputs["query"], np.float32)
    k = np.asarray(inputs["key"], np.float32)
    v = np.asarray(inputs["value"], np.float32)
    Wq = np.asarray(inputs["Wq"], np.float32)
    Wk = np.asarray(inputs["Wk"], np.float32)
    Wv = np.asarray(inputs["Wv"], np.float32)
    Wo = np.asarray(inputs["Wo"], np.float32)
    bq = np.asarray(inputs["bq"], np.float32)
    bk = np.asarray(inputs["bk"], np.float32)
    bv = np.asarray(inputs["bv"], np.float32)

    def aug_x(x):  # [s, D] -> [D+1, s] bf16 with ones row
        out = np.empty((D + 1, s), BF16NP)
        out[:D] = np.ascontiguousarray(x.T).astype(BF16NP)
        out[D] = np.ones((s,), BF16NP)
        return out

    def aug_w(W, b, dims):  # -> [D+1, HD] bf16 with bias row
        out = np.empty((D + 1, HD), BF16NP)
        out[:D] = W[dims, :].T.astype(BF16NP)
        out[D] = b[dims].astype(BF16NP)
        return out

    in_maps = []
    for c in range(NCORES):
        b = c // 4
        g = c % 4
        dims = slice(g * HD, (g + 1) * HD)
        in_maps.append({
            "xqT": aug_x(q[b]),
            "xkT": aug_x(k[b]),
            "xvT": aug_x(v[b]),
            "wqT": aug_w(Wq, bq, dims),
            "wkT": aug_w(Wk, bk, dims),
            "wvT": aug_w(Wv, bv, dims),
            "woT": np.ascontiguousarray(Wo[:, dims].T).astype(BF16NP),
        })
    return in_maps


def _ensure_ntff_hook():
    """bass_utils' axon trace path imports antenv.axon_hooks, which is
    absent from this image; synthesize it around trn_agent_boot's ctypes
    NTFF driver so neuron-profile tracing works."""
    try:
        import antenv.axon_hooks  # noqa: F401
        return
    except ImportError:
        pass
    import types

    try:
        import antenv
    except ImportError:
        return
    state = {"hook": None, "built": False}

    def _get():
        if not state["built"]:
            state["built"] = True
            try:
                from trn_agent_boot.trn_boot import _ntff_profile_via_ctypes
                state["hook"] = _ntff_profile_via_ctypes("/opt/axon/libaxon_pjrt.so")
            except Exception:
                state["hook"] = None
        return state["hook"]

    def _set(h):
        state["hook"] = h
        state["built"] = True

    mod = types.ModuleType("antenv.axon_hooks")
    mod.get_axon_ntff_profile_hook = _get
    mod.set_axon_ntff_profile_hook = _set
    sys.modules["antenv.axon_hooks"] = mod
    antenv.axon_hooks = mod


def kernel(**inputs):
    from concourse import bass_utils

    _ensure_ntff_hook()
    if "nc" not in _CACHE:
        _CACHE["nc"] = _build(S)
    nc = _CACHE["nc"]

    in_maps = make_in_maps(inputs, S)
    res = bass_utils.run_bass_kernel_spmd(
        nc,
        in_maps,
        core_ids=list(range(NCORES)),
        trace=bool(os.environ.get("KERNEL_TRACE")),
    )
    _CACHE["last_result"] = res

    bo = np.asarray(inputs["bo"], np.float32)
    attn_full = np.empty((B, H, S, S), np.float32)
    out_full = np.zeros((B, S, D), np.float32)
    for c in range(NCORES):
        b = c // 4
        g = c % 4
        attn_full[b, g * HPC:(g + 1) * HPC] = res.results[c]["attn"]
        out_full[b] += res.results[c]["outp"]
    out_full += bo
    return out_full, attn_full


# revision 11
# speedup vs baseline: 1.2733x; 1.2733x over previous
"""Trainium2 Bass kernel: 12-head MHA (B=2, S=4096, D=768) sharded over 8 cores.

Sharding: batch x head-group. Core c handles batch b=c//4 and heads
3*(c%4) .. 3*(c%4)+2 (Megatron-style column split of Wq/Wk/Wv, row split
of Wo; the Wo all-reduce is the host-side gather/sum).

Per-core device algorithm ("R2b"):
  - Projections on PE with biases folded in via an augmented ones-row in
    the contraction dim.  Q^T,K^T produced dk-major [64, S]; V produced
    seq-major [S, 192].
  - PATH-B (per head): scores^T tiles [128k, q] on PE, exp on ScalarE
    (unnormalized, bf16), A^T @ V accumulated on PE -> O_un^T [64, S].
  - PATH-A (per head): scores tiles [128q, k] on PE, exp on ScalarE with
    accum_out giving softmax row-sums for free, DVE normalize, DMA out
    full f32 attention weights (the dominant 1.6 GB output).
  - Final: per-head O_un^T @ Wo^T partial products, scaled by the
    per-(head,row) reciprocal row-sum during PSUM evacuation, summed on
    DVE, DMA out f32 partial output; host sums the 4 partials per batch
    and adds bo.
"""

import os
import sys

import numpy as np

_REPO = "/opt/trn_rl_repo"
if _REPO not in sys.path:
    sys.path.insert(0, _REPO)

import ml_dtypes  # noqa: E402

BF16NP = ml_dtypes.bfloat16

B, S, D, H, DK = 2, 4096, 768, 12, 64
HPC = 3            # heads per core
HD = HPC * DK      # 192 head dims per core
NCORES = 8
SCALE = 1.0 / 8.0  # 1/sqrt(DK)

_CACHE = {}


def _build(s=S):
    """Build and compile the per-core Bass/Tile kernel (SPMD; same NEFF on
    every core)."""
    import concourse.bacc as bacc
    import concourse.mybir as mybir
    import concourse.tile as tile

    f32 = mybir.dt.float32
    bf16 = mybir.dt.bfloat16

    nc = bacc.Bacc(
        "TRN2",
        target_bir_lowering=False,
        debug=False,
        enable_asserts=False,
        num_devices=NCORES,
    )

    t = {}
    for n in ("xqT", "xkT", "xvT"):
        t[n] = nc.dram_tensor(n, [D + 1, s], bf16, kind="ExternalInput").ap()
    for n in ("wqT", "wkT", "wvT"):
        t[n] = nc.dram_tensor(n, [D + 1, HD], bf16, kind="ExternalInput").ap()
    t["woT"] = nc.dram_tensor("woT", [HD, D], bf16, kind="ExternalInput").ap()
    t["attn"] = nc.dram_tensor("attn", [HPC, s, s], f32, kind="ExternalOutput").ap()
    t["outp"] = nc.dram_tensor("outp", [s, D], f32, kind="ExternalOutput").ap()

    with tile.TileContext(nc) as tc:
        _emit(tc, nc, t, s, mybir)

    nc.compile()
    return nc


def _emit(tc, nc, t, s, mybir):
    from contextlib import ExitStack

    f32 = mybir.dt.float32
    bf16 = mybir.dt.bfloat16
    Exp = mybir.ActivationFunctionType.Exp

    NQ5 = s // 512    # 512-wide tiles
    NQT = s // 128    # 128-wide tiles
    NKC = s // 128    # contraction chunks of 128 over seq
    BW = min(1024, s)         # PATH-B q-tile width (psum-limited)
    NB = s // BW
    AW = min(2048, s)         # PATH-A exp width over k
    NA = s // AW
    NCH = D // 128            # 6 weight/x chunks of 128 (+1 ones row)

    with ExitStack() as ctx:
        persist = ctx.enter_context(tc.tile_pool(name="persist", bufs=1))

        qt_h = [persist.tile([DK, s], bf16, name=f"qt{h}", tag=f"qt{h}") for h in range(HPC)]
        kt_h = [persist.tile([DK, s], bf16, name=f"kt{h}", tag=f"kt{h}") for h in range(HPC)]
        vsb = persist.tile([128, NKC, HD], bf16, name="vsb", tag="vsb")
        ot_h = [persist.tile([DK, s], bf16, name=f"ot{h}", tag=f"ot{h}") for h in range(HPC)]
        wo_h = [persist.tile([DK, D], bf16, name=f"wo{h}", tag=f"wo{h}") for h in range(HPC)]
        recip = persist.tile([128, HPC * NQT], f32, name="recip", tag="recip")

        for h in range(HPC):
            nc.sync.dma_start(wo_h[h][:, :], t["woT"][h * DK:(h + 1) * DK, :])

        # ---------------- projections ----------------
        with (
            tc.tile_pool(name="wch", bufs=1) as wpool,
            tc.tile_pool(name="xch", bufs=1) as xpool,
            tc.tile_pool(name="pps", bufs=2, space="PSUM") as pps,
        ):
            wch = {}
            for n in ("wqT", "wkT", "wvT"):
                chunks = []
                for c in range(NCH):
                    w = wpool.tile([128, HD], bf16, name=f"{n}{c}", tag=f"{n}{c}")
                    nc.sync.dma_start(w[:, :], t[n][c * 128:(c + 1) * 128, :])
                    chunks.append(w)
                w = wpool.tile([1, HD], bf16, name=f"{n}6", tag=f"{n}6")
                nc.sync.dma_start(w[:, :], t[n][D:D + 1, :])
                chunks.append(w)
                wch[n] = chunks

            def load_x(n):
                xs = []
                for c in range(NCH):
                    x = xpool.tile([128, s], bf16, name=f"x{c}", tag=f"x{c}")
                    nc.sync.dma_start(x[:, :], t[n][c * 128:(c + 1) * 128, :])
                    xs.append(x)
                x = xpool.tile([1, s], bf16, name="x6", tag="x6")
                nc.sync.dma_start(x[:, :], t[n][D:D + 1, :])
                xs.append(x)
                return xs

            # Q^T and K^T: dk-major [64, s] per head
            for xn, wn, out in (("xqT", "wqT", qt_h), ("xkT", "wkT", kt_h)):
                xs = load_x(xn)
                for q5 in range(NQ5):
                    qs = slice(q5 * 512, q5 * 512 + 512)
                    for h in range(HPC):
                        ps = pps.tile([DK, 512], f32, name="psqk", tag="psqk")
                        for c in range(NCH + 1):
                            nc.tensor.matmul(
                                ps[:, :],
                                wch[wn][c][:, h * DK:(h + 1) * DK],
                                xs[c][:, qs],
                                start=(c == 0),
                                stop=(c == NCH),
                            )
                        nc.vector.tensor_copy(out[h][:, qs], ps[:, :])

            # V: seq-major [s, 192]
            xs = load_x("xvT")
            for qt in range(NQT):
                qs = slice(qt * 128, qt * 128 + 128)
                ps = pps.tile([128, HD], f32, name="psv", tag="psv")
                for c in range(NCH + 1):
                    nc.tensor.matmul(
                        ps[:, :],
                        xs[c][:, qs],
                        wch["wvT"][c][:, :],
                        start=(c == 0),
                        stop=(c == NCH),
                    )
                nc.vector.tensor_copy(vsb[:, qt, :], ps[:, :])

        # ---------------- per-head attention ----------------
        for h in range(HPC):
            # PATH-B: scores^T -> exp(bf16) -> A_un^T @ V -> O_un^T
            with (
                tc.tile_pool(name=f"bps{h}", bufs=1, space="PSUM") as bps,
                tc.tile_pool(name=f"but{h}", bufs=2) as butp,
            ):
                for q2 in range(NB):
                    q2s = slice(q2 * BW, (q2 + 1) * BW)
                    avp = bps.tile([DK, BW], f32, name="avp", tag="avp")
                    for kc in range(NKC):
                        stp = bps.tile([128, BW], f32, name="stp", tag="stp",
                                       bufs=2)
                        for sub in range(BW // 512):
                            qs = slice(q2 * BW + sub * 512, q2 * BW + sub * 512 + 512)
                            nc.tensor.matmul(
                                stp[:, sub * 512:(sub + 1) * 512],
                                kt_h[h][:, kc * 128:(kc + 1) * 128],
                                qt_h[h][:, qs],
                                start=True,
                                stop=True,
                            )
                        ut = butp.tile([128, BW], bf16, name="ut", tag="ut",
                                       bufs=3)
                        nc.scalar.activation(ut[:, :], stp[:, :], Exp, scale=SCALE)
                        for sub in range(BW // 512):
                            nc.tensor.matmul(
                                avp[:, sub * 512:(sub + 1) * 512],
                                vsb[:, kc, h * DK:(h + 1) * DK],
                                ut[:, sub * 512:(sub + 1) * 512],
                                start=(kc == 0),
                                stop=(kc == NKC - 1),
                            )
                    nc.vector.tensor_copy(ot_h[h][:, q2s], avp[:, :])

            # PATH-A: scores -> exp (+rowsum via accum_out) -> normalize -> DMA
            with (
                tc.tile_pool(name=f"aps{h}", bufs=2, space="PSUM") as aps,
                tc.tile_pool(name=f"au{h}", bufs=2) as aup,
                tc.tile_pool(name=f"aa{h}", bufs=2) as aap,
                tc.tile_pool(name=f"ark{h}", bufs=2) as arkp,
            ):
                for qt in range(NQT):
                    qs = slice(qt * 128, qt * 128 + 128)
                    u = aup.tile([128, s], f32, name="u", tag="u")
                    rk = arkp.tile([128, max(NA, 2)], f32, name="rk", tag="rk")
                    for half in range(NA):
                        sp = aps.tile([128, AW], f32, name="sp", tag="sp")
                        for sub in range(AW // 512):
                            ks = slice(half * AW + sub * 512,
                                       half * AW + sub * 512 + 512)
                            nc.tensor.matmul(
                                sp[:, sub * 512:(sub + 1) * 512],
                                qt_h[h][:, qs],
                                kt_h[h][:, ks],
                                start=True,
                                stop=True,
                            )
                        nc.scalar.activation(
                            u[:, half * AW:(half + 1) * AW],
                            sp[:, :],
                            Exp,
                            scale=SCALE,
                            accum_out=rk[:, half:half + 1],
                        )
                    rsum = arkp.tile([128, 1], f32, name="rsum", tag="rsum")
                    if NA == 2:
                        nc.vector.tensor_add(rsum[:, :], rk[:, 0:1], rk[:, 1:2])
                    elif NA == 1:
                        nc.vector.tensor_copy(rsum[:, :], rk[:, 0:1])
                    else:
                        nc.vector.tensor_reduce(
                            rsum[:, :], rk[:, 0:NA],
                            axis=mybir.AxisListType.X, op=mybir.AluOpType.add,
                        )
                    rc = recip[:, h * NQT + qt:h * NQT + qt + 1]
                    nc.vector.reciprocal(rc, rsum[:, :])
                    a = aap.tile([128, s], f32, name="a", tag="a")
                    nc.vector.tensor_scalar_mul(a[:, :], u[:, :], rc)
                    nc.sync.dma_start(t["attn"][h, qs, :], a[:, :])

        # ---------------- output projection (partial) ----------------
        with (
            tc.tile_pool(name="fps", bufs=2, space="PSUM") as fps,
            tc.tile_pool(name="facc", bufs=2) as faccp,
            tc.tile_pool(name="ftmp", bufs=2) as ftmpp,
        ):
            for qt in range(NQT):
                qs = slice(qt * 128, qt * 128 + 128)
                acc = faccp.tile([128, D], f32, name="acc", tag="acc")
                for h in range(HPC):
                    pf = fps.tile([128, D], f32, name="pf", tag="pf")
                    nc.tensor.matmul(pf[:, 0:512], ot_h[h][:, qs],
                                     wo_h[h][:, 0:512], start=True, stop=True)
                    nc.tensor.matmul(pf[:, 512:D], ot_h[h][:, qs],
                                     wo_h[h][:, 512:D], start=True, stop=True)
                    rc = recip[:, h * NQT + qt:h * NQT + qt + 1]
                    if h == 0:
                        nc.vector.tensor_scalar_mul(acc[:, :], pf[:, :], rc)
                    else:
                        tmp = ftmpp.tile([128, D], f32, name="tmp", tag="tmp")
                        nc.vector.tensor_scalar_mul(tmp[:, :], pf[:, :], rc)
                        nc.vector.tensor_add(acc[:, :], acc[:, :], tmp[:, :])
                nc.sync.dma_start(t["outp"][qs, :], acc[:, :])


def make_in_maps(inputs, s=S):
    """Shard + lay out the full inputs for the 8 cores."""
    q = np.asarray(inputs["query"], np.float32)
    k = np.asarray(inputs["key"], np.float32)
    v = np.asarray(inputs["value"], np.float32)
    Wq = np.asarray(inputs["Wq"], np.float32)
    Wk = np.asarray(inputs["Wk"], np.float32)
    Wv = np.asarray(inputs["Wv"], np.float32)
    Wo = np.asarray(inputs["Wo"], np.float32)
    bq = np.asarray(inputs["bq"], np.float32)
    bk = np.asarray(inputs["bk"], np.float32)
    bv = np.asarray(inputs["bv"], np.float32)

    def aug_x(x):  # [s, D] -> [D+1, s] bf16 with ones row
        out = np.empty((D + 1, s), BF16NP)
        out[:D] = np.ascontiguousarray(x.T).astype(BF16NP)
        out[D] = np.ones((s,), BF16NP)
        return out

    def aug_w(W, b, dims):  # -> [D+1, HD] bf16 with bias row
        out = np.empty((D + 1, HD), BF16NP)
        out[:D] = W[dims, :].T.astype(BF16NP)
        out[D] = b[dims].astype(BF16NP)
        return out

    in_maps = []
    for c in range(NCORES):
        b = c // 4
        g = c % 4
        dims = slice(g * HD, (g + 1) * HD)
        in_maps.append({
            "xqT": aug_x(q[b]),
            "xkT": aug_x(k[b]),
            "xvT": aug_x(v[b]),
            "wqT": aug_w(Wq, bq, dims),
            "wkT": aug_w(Wk, bk, dims),
            "wvT": aug_w(Wv, bv, dims),
            "woT": np.ascontiguousarray(Wo[:, dims].T).astype(BF16NP),
        })
    return in_maps


def _ensure_ntff_hook():
    """bass_utils' axon trace path imports antenv.axon_hooks, which is
    absent from this image; synthesize it around trn_agent_boot's ctypes
    NTFF driver so neuron-profile tracing works."""
    try:
        import antenv.axon_hooks  # noqa: F401
        return
    except ImportError:
        pass
    import types

    try:
        import antenv
    except ImportError:
        return
    state = {"hook": None, "built": False}

    def _get():
        if not state["built"]:
            state["built"] = True
            try:
                from trn_agent_boot.trn_boot import _ntff_profile_via_ctypes
                state["hook"] = _ntff_profile_via_ctypes("/opt/axon/libaxon_pjrt.so")
            except Exception:
                state["hook"] = None
        return state["hook"]

    def _set(h):
        state["hook"] = h
        state["built"] = True

    mod = types.ModuleType("antenv.axon_hooks")
    mod.get_axon_ntff_profile_hook = _get
    mod.set_axon_ntff_profile_hook = _set
    sys.modules["antenv.axon_hooks"] = mod
    antenv.axon_hooks = mod


def kernel(**inputs):
    from concourse import bass_utils

    _ensure_ntff_hook()
    if "nc" not in _CACHE:
        _CACHE["nc"] = _build(S)
    nc = _CACHE["nc"]

    in_maps = make_in_maps(inputs, S)
    res = bass_utils.run_bass_kernel_spmd(
        nc,
        in_maps,
        core_ids=list(range(NCORES)),
        trace=bool(os.environ.get("KERNEL_TRACE")),
    )
    _CACHE["last_result"] = res

    bo = np.asarray(inputs["bo"], np.float32)
    attn_full = np.empty((B, H, S, S), np.float32)
    out_full = np.zeros((B, S, D), np.float32)
    for c in range(NCORES):
        b = c // 4
        g = c % 4
        attn_full[b, g * HPC:(g + 1) * HPC] = res.results[c]["attn"]
        out_full[b] += res.results[c]["outp"]
    out_full += bo
    return out_full, attn_full


# revision 12
# speedup vs baseline: 1.4350x; 1.1269x over previous
"""Trainium2 Bass kernel: 12-head MHA (B=2, S=4096, D=768) sharded over 8 cores.

Sharding: batch x head-group. Core c handles batch b=c//4 and heads
3*(c%4) .. 3*(c%4)+2 (Megatron-style column split of Wq/Wk/Wv, row split
of Wo; the Wo all-reduce is the host-side gather/sum).

Per-core device algorithm ("R2b"):
  - Projections on PE with biases folded in via an augmented ones-row in
    the contraction dim.  Q^T,K^T produced dk-major [64, S]; V produced
    seq-major [S, 192].
  - PATH-B (per head): scores^T tiles [128k, q] on PE, exp on ScalarE
    (unnormalized, bf16), A^T @ V accumulated on PE -> O_un^T [64, S].
  - PATH-A (per head): scores tiles [128q, k] on PE, exp on ScalarE with
    accum_out giving softmax row-sums for free, DVE normalize, DMA out
    full f32 attention weights (the dominant 1.6 GB output).
  - Final: per-head O_un^T @ Wo^T partial products, scaled by the
    per-(head,row) reciprocal row-sum during PSUM evacuation, summed on
    DVE, DMA out f32 partial output; host sums the 4 partials per batch
    and adds bo.
"""

import os
import sys

import numpy as np

_REPO = "/opt/trn_rl_repo"
if _REPO not in sys.path:
    sys.path.insert(0, _REPO)

import ml_dtypes  # noqa: E402

BF16NP = ml_dtypes.bfloat16

B, S, D, H, DK = 2, 4096, 768, 12, 64
HPC = 3            # heads per core
HD = HPC * DK      # 192 head dims per core
NCORES = 8
SCALE = 1.0 / 8.0  # 1/sqrt(DK)

_CACHE = {}


def _build(s=S):
    """Build and compile the per-core Bass/Tile kernel (SPMD; same NEFF on
    every core)."""
    import concourse.bacc as bacc
    import concourse.mybir as mybir
    import concourse.tile as tile

    f32 = mybir.dt.float32
    bf16 = mybir.dt.bfloat16

    nc = bacc.Bacc(
        "TRN2",
        target_bir_lowering=False,
        debug=False,
        enable_asserts=False,
        num_devices=NCORES,
    )

    t = {}
    for n in ("xqT", "xkT", "xvT"):
        t[n] = nc.dram_tensor(n, [D + 1, s], bf16, kind="ExternalInput").ap()
    for n in ("wqT", "wkT", "wvT"):
        t[n] = nc.dram_tensor(n, [D + 1, HD], bf16, kind="ExternalInput").ap()
    t["woT"] = nc.dram_tensor("woT", [HD, D], bf16, kind="ExternalInput").ap()
    t["attn"] = nc.dram_tensor("attn", [HPC, s, s], f32, kind="ExternalOutput").ap()
    t["outp"] = nc.dram_tensor("outp", [s, D], f32, kind="ExternalOutput").ap()

    with tile.TileContext(nc) as tc:
        _emit(tc, nc, t, s, mybir)

    nc.compile()
    return nc


def _emit(tc, nc, t, s, mybir):
    from contextlib import ExitStack

    f32 = mybir.dt.float32
    bf16 = mybir.dt.bfloat16
    Exp = mybir.ActivationFunctionType.Exp

    NQ5 = s // 512    # 512-wide tiles
    NQT = s // 128    # 128-wide tiles
    NKC = s // 128    # contraction chunks of 128 over seq
    BW = min(1024, s)         # PATH-B q-tile width (psum-limited)
    NB = s // BW
    AW = min(2048, s)         # PATH-A exp width over k
    NA = s // AW
    NCH = D // 128            # 6 weight/x chunks of 128 (+1 ones row)

    with ExitStack() as ctx:
        persist = ctx.enter_context(tc.tile_pool(name="persist", bufs=1))

        qt_h = [persist.tile([128, s], bf16, name=f"qt{h}", tag=f"qt{h}") for h in range(HPC)]
        kt_h = [persist.tile([128, s], bf16, name=f"kt{h}", tag=f"kt{h}") for h in range(HPC)]
        vsb = persist.tile([128, NKC, HD], bf16, name="vsb", tag="vsb")
        ot_h = [persist.tile([DK, s], bf16, name=f"ot{h}", tag=f"ot{h}") for h in range(HPC)]
        wo_h = [persist.tile([DK, D], bf16, name=f"wo{h}", tag=f"wo{h}") for h in range(HPC)]
        recip = persist.tile([128, HPC * NQT], f32, name="recip", tag="recip")

        for h in range(HPC):
            nc.sync.dma_start(wo_h[h][:, :], t["woT"][h * DK:(h + 1) * DK, :])

        # ---------------- projections ----------------
        with (
            tc.tile_pool(name="wch", bufs=1) as wpool,
            tc.tile_pool(name="xch", bufs=1) as xpool,
            tc.tile_pool(name="pps", bufs=2, space="PSUM") as pps,
        ):
            wch = {}
            for n in ("wqT", "wkT", "wvT"):
                chunks = []
                for c in range(NCH):
                    w = wpool.tile([128, HD], bf16, name=f"{n}{c}", tag=f"{n}{c}")
                    nc.sync.dma_start(w[:, :], t[n][c * 128:(c + 1) * 128, :])
                    chunks.append(w)
                w = wpool.tile([1, HD], bf16, name=f"{n}6", tag=f"{n}6")
                nc.sync.dma_start(w[:, :], t[n][D:D + 1, :])
                chunks.append(w)
                wch[n] = chunks

            def load_x(n):
                xs = []
                for c in range(NCH):
                    x = xpool.tile([128, s], bf16, name=f"x{c}", tag=f"x{c}")
                    nc.sync.dma_start(x[:, :], t[n][c * 128:(c + 1) * 128, :])
                    xs.append(x)
                x = xpool.tile([1, s], bf16, name="x6", tag="x6")
                nc.sync.dma_start(x[:, :], t[n][D:D + 1, :])
                xs.append(x)
                return xs

            # Q^T and K^T: dk-major [64, s] per head
            for xn, wn, out in (("xqT", "wqT", qt_h), ("xkT", "wkT", kt_h)):
                xs = load_x(xn)
                for q5 in range(NQ5):
                    qs = slice(q5 * 512, q5 * 512 + 512)
                    for h in range(HPC):
                        psA = pps.tile([128, 512], f32, name="psqkA", tag="psqkA")
                        psB = pps.tile([128, 512], f32, name="psqkB", tag="psqkB")
                        for c in range(NCH + 1):
                            nc.tensor.matmul(
                                psA[0:DK, :],
                                wch[wn][c][:, h * DK:(h + 1) * DK],
                                xs[c][:, qs],
                                start=(c == 0), stop=(c == NCH),
                                tile_position=(0, 0),
                            )
                            nc.tensor.matmul(
                                psB[DK:128, :],
                                wch[wn][c][:, h * DK:(h + 1) * DK],
                                xs[c][:, qs],
                                start=(c == 0), stop=(c == NCH),
                                tile_position=(0, 64),
                            )
                        nc.vector.tensor_copy(out[h][0:DK, qs], psA[0:DK, :])
                        nc.vector.tensor_copy(out[h][DK:128, qs], psB[DK:128, :])

            # V: seq-major [s, 192]
            xs = load_x("xvT")
            for qt in range(NQT):
                qs = slice(qt * 128, qt * 128 + 128)
                ps = pps.tile([128, HD], f32, name="psv", tag="psv")
                for c in range(NCH + 1):
                    nc.tensor.matmul(
                        ps[:, :],
                        xs[c][:, qs],
                        wch["wvT"][c][:, :],
                        start=(c == 0),
                        stop=(c == NCH),
                    )
                nc.vector.tensor_copy(vsb[:, qt, :], ps[:, :])

        # ---------------- per-head attention ----------------
        for h in range(HPC):
            # PATH-B: scores^T -> exp(bf16) -> A_un^T @ V -> O_un^T
            with (
                tc.tile_pool(name=f"bps{h}", bufs=1, space="PSUM") as bps,
                tc.tile_pool(name=f"but{h}", bufs=2) as butp,
            ):
                for q2 in range(NB):
                    q2s = slice(q2 * BW, (q2 + 1) * BW)
                    avp = bps.tile([DK, BW], f32, name="avp", tag="avp")
                    for kc in range(NKC):
                        stp = bps.tile([128, BW], f32, name="stp", tag="stp",
                                       bufs=2)
                        for sub in range(BW // 512):
                            qs = slice(q2 * BW + sub * 512, q2 * BW + sub * 512 + 512)
                            rp = slice(0, DK) if sub % 2 == 0 else slice(DK, 128)
                            nc.tensor.matmul(
                                stp[:, sub * 512:(sub + 1) * 512],
                                kt_h[h][rp, kc * 128:(kc + 1) * 128],
                                qt_h[h][rp, qs],
                                start=True,
                                stop=True,
                            )
                        ut = butp.tile([128, BW], bf16, name="ut", tag="ut",
                                       bufs=3)
                        nc.scalar.activation(ut[:, :], stp[:, :], Exp, scale=SCALE)
                        for sub in range(BW // 512):
                            nc.tensor.matmul(
                                avp[:, sub * 512:(sub + 1) * 512],
                                vsb[:, kc, h * DK:(h + 1) * DK],
                                ut[:, sub * 512:(sub + 1) * 512],
                                start=(kc == 0),
                                stop=(kc == NKC - 1),
                            )
                    nc.vector.tensor_copy(ot_h[h][:, q2s], avp[:, :])

            # PATH-A: scores -> exp (+rowsum via accum_out) -> normalize -> DMA
            with (
                tc.tile_pool(name=f"aps{h}", bufs=2, space="PSUM") as aps,
                tc.tile_pool(name=f"au{h}", bufs=2) as aup,
                tc.tile_pool(name=f"aa{h}", bufs=2) as aap,
                tc.tile_pool(name=f"ark{h}", bufs=2) as arkp,
            ):
                for qt in range(NQT):
                    qs = slice(qt * 128, qt * 128 + 128)
                    u = aup.tile([128, s], f32, name="u", tag="u")
                    rk = arkp.tile([128, max(NA, 2)], f32, name="rk", tag="rk")
                    for half in range(NA):
                        sp = aps.tile([128, AW], f32, name="sp", tag="sp")
                        for sub in range(AW // 512):
                            ks = slice(half * AW + sub * 512,
                                       half * AW + sub * 512 + 512)
                            rp = slice(0, DK) if sub % 2 == 0 else slice(DK, 128)
                            nc.tensor.matmul(
                                sp[:, sub * 512:(sub + 1) * 512],
                                qt_h[h][rp, qs],
                                kt_h[h][rp, ks],
                                start=True,
                                stop=True,
                            )
                        nc.scalar.activation(
                            u[:, half * AW:(half + 1) * AW],
                            sp[:, :],
                            Exp,
                            scale=SCALE,
                            accum_out=rk[:, half:half + 1],
                        )
                    rsum = arkp.tile([128, 1], f32, name="rsum", tag="rsum")
                    if NA == 2:
                        nc.vector.tensor_add(rsum[:, :], rk[:, 0:1], rk[:, 1:2])
                    elif NA == 1:
                        nc.vector.tensor_copy(rsum[:, :], rk[:, 0:1])
                    else:
                        nc.vector.tensor_reduce(
                            rsum[:, :], rk[:, 0:NA],
                            axis=mybir.AxisListType.X, op=mybir.AluOpType.add,
                        )
                    rc = recip[:, h * NQT + qt:h * NQT + qt + 1]
                    nc.vector.reciprocal(rc, rsum[:, :])
                    a = aap.tile([128, s], f32, name="a", tag="a")
                    nc.vector.tensor_scalar_mul(a[:, :], u[:, :], rc)
                    nc.sync.dma_start(t["attn"][h, qs, :], a[:, :])

        # ---------------- output projection (partial) ----------------
        with (
            tc.tile_pool(name="fps", bufs=2, space="PSUM") as fps,
            tc.tile_pool(name="facc", bufs=2) as faccp,
            tc.tile_pool(name="ftmp", bufs=2) as ftmpp,
        ):
            for qt in range(NQT):
                qs = slice(qt * 128, qt * 128 + 128)
                acc = faccp.tile([128, D], f32, name="acc", tag="acc")
                for h in range(HPC):
                    pf = fps.tile([128, D], f32, name="pf", tag="pf")
                    nc.tensor.matmul(pf[:, 0:512], ot_h[h][:, qs],
                                     wo_h[h][:, 0:512], start=True, stop=True)
                    nc.tensor.matmul(pf[:, 512:D], ot_h[h][:, qs],
                                     wo_h[h][:, 512:D], start=True, stop=True)
                    rc = recip[:, h * NQT + qt:h * NQT + qt + 1]
                    if h == 0:
                        nc.vector.tensor_scalar_mul(acc[:, :], pf[:, :], rc)
                    else:
                        tmp = ftmpp.tile([128, D], f32, name="tmp", tag="tmp")
                        nc.vector.tensor_scalar_mul(tmp[:, :], pf[:, :], rc)
                        nc.vector.tensor_add(acc[:, :], acc[:, :], tmp[:, :])
                nc.sync.dma_start(t["outp"][qs, :], acc[:, :])


def make_in_maps(inputs, s=S):
    """Shard + lay out the full inputs for the 8 cores."""
    q = np.asarray(inputs["query"], np.float32)
    k = np.asarray(inputs["key"], np.float32)
    v = np.asarray(inputs["value"], np.float32)
    Wq = np.asarray(inputs["Wq"], np.float32)
    Wk = np.asarray(inputs["Wk"], np.float32)
    Wv = np.asarray(inputs["Wv"], np.float32)
    Wo = np.asarray(inputs["Wo"], np.float32)
    bq = np.asarray(inputs["bq"], np.float32)
    bk = np.asarray(inputs["bk"], np.float32)
    bv = np.asarray(inputs["bv"], np.float32)

    def aug_x(x):  # [s, D] -> [D+1, s] bf16 with ones row
        out = np.empty((D + 1, s), BF16NP)
        out[:D] = np.ascontiguousarray(x.T).astype(BF16NP)
        out[D] = np.ones((s,), BF16NP)
        return out

    def aug_w(W, b, dims):  # -> [D+1, HD] bf16 with bias row
        out = np.empty((D + 1, HD), BF16NP)
        out[:D] = W[dims, :].T.astype(BF16NP)
        out[D] = b[dims].astype(BF16NP)
        return out

    in_maps = []
    for c in range(NCORES):
        b = c // 4
        g = c % 4
        dims = slice(g * HD, (g + 1) * HD)
        in_maps.append({
            "xqT": aug_x(q[b]),
            "xkT": aug_x(k[b]),
            "xvT": aug_x(v[b]),
            "wqT": aug_w(Wq, bq, dims),
            "wkT": aug_w(Wk, bk, dims),
            "wvT": aug_w(Wv, bv, dims),
            "woT": np.ascontiguousarray(Wo[:, dims].T).astype(BF16NP),
        })
    return in_maps


def _ensure_ntff_hook():
    """bass_utils' axon trace path imports antenv.axon_hooks, which is
    absent from this image; synthesize it around trn_agent_boot's ctypes
    NTFF driver so neuron-profile tracing works."""
    try:
        import antenv.axon_hooks  # noqa: F401
        return
    except ImportError:
        pass
    import types

    try:
        import antenv
    except ImportError:
        return
    state = {"hook": None, "built": False}

    def _get():
        if not state["built"]:
            state["built"] = True
            try:
                from trn_agent_boot.trn_boot import _ntff_profile_via_ctypes
                state["hook"] = _ntff_profile_via_ctypes("/opt/axon/libaxon_pjrt.so")
            except Exception:
                state["hook"] = None
        return state["hook"]

    def _set(h):
        state["hook"] = h
        state["built"] = True

    mod = types.ModuleType("antenv.axon_hooks")
    mod.get_axon_ntff_profile_hook = _get
    mod.set_axon_ntff_profile_hook = _set
    sys.modules["antenv.axon_hooks"] = mod
    antenv.axon_hooks = mod


def kernel(**inputs):
    from concourse import bass_utils

    _ensure_ntff_hook()
    if "nc" not in _CACHE:
        _CACHE["nc"] = _build(S)
    nc = _CACHE["nc"]

    in_maps = make_in_maps(inputs, S)
    res = bass_utils.run_bass_kernel_spmd(
        nc,
        in_maps,
        core_ids=list(range(NCORES)),
        trace=bool(os.environ.get("KERNEL_TRACE")),
    )
    _CACHE["last_result"] = res

    bo = np.asarray(inputs["bo"], np.float32)
    attn_full = np.empty((B, H, S, S), np.float32)
    out_full = np.zeros((B, S, D), np.float32)
    for c in range(NCORES):
        b = c // 4
        g = c % 4
        attn_full[b, g * HPC:(g + 1) * HPC] = res.results[c]["attn"]
        out_full[b] += res.results[c]["outp"]
    out_full += bo
    return out_full, attn_full


# revision 13
# speedup vs baseline: 1.4890x; 1.0376x over previous
"""Trainium2 Bass kernel: 12-head MHA (B=2, S=4096, D=768) sharded over 8 cores.

Sharding: batch x head-group. Core c handles batch b=c//4 and heads
3*(c%4) .. 3*(c%4)+2 (Megatron-style column split of Wq/Wk/Wv, row split
of Wo; the Wo all-reduce is the host-side gather/sum).

Per-core device algorithm ("R2b"):
  - Projections on PE with biases folded in via an augmented ones-row in
    the contraction dim.  Q^T,K^T produced dk-major [64, S]; V produced
    seq-major [S, 192].
  - PATH-B (per head): scores^T tiles [128k, q] on PE, exp on ScalarE
    (unnormalized, bf16), A^T @ V accumulated on PE -> O_un^T [64, S].
  - PATH-A (per head): scores tiles [128q, k] on PE, exp on ScalarE with
    accum_out giving softmax row-sums for free, DVE normalize, DMA out
    full f32 attention weights (the dominant 1.6 GB output).
  - Final: per-head O_un^T @ Wo^T partial products, scaled by the
    per-(head,row) reciprocal row-sum during PSUM evacuation, summed on
    DVE, DMA out f32 partial output; host sums the 4 partials per batch
    and adds bo.
"""

import os
import sys

import numpy as np

_REPO = "/opt/trn_rl_repo"
if _REPO not in sys.path:
    sys.path.insert(0, _REPO)

import ml_dtypes  # noqa: E402

BF16NP = ml_dtypes.bfloat16

B, S, D, H, DK = 2, 4096, 768, 12, 64
HPC = 3            # heads per core
HD = HPC * DK      # 192 head dims per core
NCORES = 8
SCALE = 1.0 / 8.0  # 1/sqrt(DK)

_CACHE = {}


def _build(s=S):
    """Build and compile the per-core Bass/Tile kernel (SPMD; same NEFF on
    every core)."""
    import concourse.bacc as bacc
    import concourse.mybir as mybir
    import concourse.tile as tile

    f32 = mybir.dt.float32
    bf16 = mybir.dt.bfloat16

    nc = bacc.Bacc(
        "TRN2",
        target_bir_lowering=False,
        debug=False,
        enable_asserts=False,
        num_devices=NCORES,
    )

    t = {}
    for n in ("xqT", "xkT", "xvT"):
        t[n] = nc.dram_tensor(n, [D + 1, s], bf16, kind="ExternalInput").ap()
    for n in ("wqT", "wkT", "wvT"):
        t[n] = nc.dram_tensor(n, [D + 1, HD], bf16, kind="ExternalInput").ap()
    t["woT"] = nc.dram_tensor("woT", [HD, D], bf16, kind="ExternalInput").ap()
    t["attn"] = nc.dram_tensor("attn", [HPC, s, s], f32, kind="ExternalOutput").ap()
    t["outp"] = nc.dram_tensor("outp", [s, D], f32, kind="ExternalOutput").ap()

    with tile.TileContext(nc) as tc:
        _emit(tc, nc, t, s, mybir)

    nc.compile()
    return nc


def _emit(tc, nc, t, s, mybir):
    from contextlib import ExitStack

    f32 = mybir.dt.float32
    bf16 = mybir.dt.bfloat16
    Exp = mybir.ActivationFunctionType.Exp

    NQ5 = s // 512    # 512-wide tiles
    NQT = s // 128    # 128-wide tiles
    NKC = s // 128    # contraction chunks of 128 over seq
    BW = min(1024, s)         # PATH-B q-tile width (psum-limited)
    NB = s // BW
    AW = min(2048, s)         # PATH-A exp width over k
    NA = s // AW
    NCH = D // 128            # 6 weight/x chunks of 128 (+1 ones row)

    with ExitStack() as ctx:
        persist = ctx.enter_context(tc.tile_pool(name="persist", bufs=1))

        qt_h = [persist.tile([128, s], bf16, name=f"qt{h}", tag=f"qt{h}") for h in range(HPC)]
        kt_h = [persist.tile([128, s], bf16, name=f"kt{h}", tag=f"kt{h}") for h in range(HPC)]
        vsb = persist.tile([128, NKC, HD], bf16, name="vsb", tag="vsb")
        ot_h = [persist.tile([128, s], bf16, name=f"ot{h}", tag=f"ot{h}") for h in range(HPC)]
        wo_h = [persist.tile([128, D], bf16, name=f"wo{h}", tag=f"wo{h}") for h in range(HPC)]
        recip = persist.tile([128, HPC * NQT], f32, name="recip", tag="recip")

        for h in range(HPC):
            nc.sync.dma_start(wo_h[h][0:DK, :], t["woT"][h * DK:(h + 1) * DK, :])
            nc.sync.dma_start(wo_h[h][DK:128, :], t["woT"][h * DK:(h + 1) * DK, :])

        # ---------------- projections ----------------
        with (
            tc.tile_pool(name="wch", bufs=1) as wpool,
            tc.tile_pool(name="xch", bufs=1) as xpool,
            tc.tile_pool(name="pps", bufs=2, space="PSUM") as pps,
        ):
            wch = {}
            for n in ("wqT", "wkT", "wvT"):
                chunks = []
                for c in range(NCH):
                    w = wpool.tile([128, HD], bf16, name=f"{n}{c}", tag=f"{n}{c}")
                    nc.sync.dma_start(w[:, :], t[n][c * 128:(c + 1) * 128, :])
                    chunks.append(w)
                w = wpool.tile([1, HD], bf16, name=f"{n}6", tag=f"{n}6")
                nc.sync.dma_start(w[:, :], t[n][D:D + 1, :])
                chunks.append(w)
                wch[n] = chunks

            def load_x(n):
                xs = []
                for c in range(NCH):
                    x = xpool.tile([128, s], bf16, name=f"x{c}", tag=f"x{c}")
                    nc.sync.dma_start(x[:, :], t[n][c * 128:(c + 1) * 128, :])
                    xs.append(x)
                x = xpool.tile([1, s], bf16, name="x6", tag="x6")
                nc.sync.dma_start(x[:, :], t[n][D:D + 1, :])
                xs.append(x)
                return xs

            # Q^T and K^T: dk-major [64, s] per head
            for xn, wn, out in (("xqT", "wqT", qt_h), ("xkT", "wkT", kt_h)):
                xs = load_x(xn)
                for q5 in range(NQ5):
                    qs = slice(q5 * 512, q5 * 512 + 512)
                    for h in range(HPC):
                        psA = pps.tile([128, 512], f32, name="psqkA", tag="psqkA")
                        psB = pps.tile([128, 512], f32, name="psqkB", tag="psqkB")
                        for c in range(NCH + 1):
                            nc.tensor.matmul(
                                psA[0:DK, :],
                                wch[wn][c][:, h * DK:(h + 1) * DK],
                                xs[c][:, qs],
                                start=(c == 0), stop=(c == NCH),
                                tile_position=(0, 0),
                            )
                            nc.tensor.matmul(
                                psB[DK:128, :],
                                wch[wn][c][:, h * DK:(h + 1) * DK],
                                xs[c][:, qs],
                                start=(c == 0), stop=(c == NCH),
                                tile_position=(0, 64),
                            )
                        nc.vector.tensor_copy(out[h][0:DK, qs], psA[0:DK, :])
                        nc.vector.tensor_copy(out[h][DK:128, qs], psB[DK:128, :])

            # V: seq-major [s, 192]
            xs = load_x("xvT")
            for qt in range(NQT):
                qs = slice(qt * 128, qt * 128 + 128)
                ps = pps.tile([128, HD], f32, name="psv", tag="psv")
                for c in range(NCH + 1):
                    nc.tensor.matmul(
                        ps[:, :],
                        xs[c][:, qs],
                        wch["wvT"][c][:, :],
                        start=(c == 0),
                        stop=(c == NCH),
                    )
                nc.vector.tensor_copy(vsb[:, qt, :], ps[:, :])

        # ---------------- per-head attention ----------------
        for h in range(HPC):
            # PATH-B: scores^T -> exp(bf16) -> A_un^T @ V -> O_un^T
            with (
                tc.tile_pool(name=f"bps{h}", bufs=1, space="PSUM") as bps,
                tc.tile_pool(name=f"but{h}", bufs=2) as butp,
            ):
                for q2 in range(NB):
                    avA = bps.tile([128, BW // 2], f32, name="avA", tag="avA")
                    avB = bps.tile([128, BW // 2], f32, name="avB", tag="avB")
                    for kc in range(NKC):
                        stp = bps.tile([128, BW], f32, name="stp", tag="stp",
                                       bufs=2)
                        for sub in range(BW // 512):
                            qs = slice(q2 * BW + sub * 512, q2 * BW + sub * 512 + 512)
                            rp = slice(0, DK) if sub % 2 == 0 else slice(DK, 128)
                            nc.tensor.matmul(
                                stp[:, sub * 512:(sub + 1) * 512],
                                kt_h[h][rp, kc * 128:(kc + 1) * 128],
                                qt_h[h][rp, qs],
                                start=True,
                                stop=True,
                            )
                        ut = butp.tile([128, BW], bf16, name="ut", tag="ut",
                                       bufs=3)
                        nc.scalar.activation(ut[:, :], stp[:, :], Exp, scale=SCALE)
                        for sub, avt in ((0, avA), (1, avB)):
                            rp = slice(0, DK) if sub == 0 else slice(DK, 128)
                            nc.tensor.matmul(
                                avt[rp, :],
                                vsb[:, kc, h * DK:(h + 1) * DK],
                                ut[:, sub * 512:(sub + 1) * 512],
                                start=(kc == 0),
                                stop=(kc == NKC - 1),
                                tile_position=(0, 0 if sub == 0 else 64),
                            )
                    nc.vector.tensor_copy(
                        ot_h[h][0:DK, q2 * BW:q2 * BW + 512], avA[0:DK, :])
                    nc.vector.tensor_copy(
                        ot_h[h][DK:128, q2 * BW + 512:(q2 + 1) * BW],
                        avB[DK:128, :])

            # PATH-A: scores -> exp (+rowsum via accum_out) -> normalize -> DMA
            with (
                tc.tile_pool(name=f"aps{h}", bufs=2, space="PSUM") as aps,
                tc.tile_pool(name=f"au{h}", bufs=2) as aup,
                tc.tile_pool(name=f"aa{h}", bufs=2) as aap,
                tc.tile_pool(name=f"ark{h}", bufs=2) as arkp,
            ):
                for qt in range(NQT):
                    qs = slice(qt * 128, qt * 128 + 128)
                    u = aup.tile([128, s], f32, name="u", tag="u")
                    rk = arkp.tile([128, max(NA, 2)], f32, name="rk", tag="rk")
                    for half in range(NA):
                        sp = aps.tile([128, AW], f32, name="sp", tag="sp")
                        for sub in range(AW // 512):
                            ks = slice(half * AW + sub * 512,
                                       half * AW + sub * 512 + 512)
                            rp = slice(0, DK) if sub % 2 == 0 else slice(DK, 128)
                            nc.tensor.matmul(
                                sp[:, sub * 512:(sub + 1) * 512],
                                qt_h[h][rp, qs],
                                kt_h[h][rp, ks],
                                start=True,
                                stop=True,
                            )
                        nc.scalar.activation(
                            u[:, half * AW:(half + 1) * AW],
                            sp[:, :],
                            Exp,
                            scale=SCALE,
                            accum_out=rk[:, half:half + 1],
                        )
                    rsum = arkp.tile([128, 1], f32, name="rsum", tag="rsum")
                    if NA == 2:
                        nc.vector.tensor_add(rsum[:, :], rk[:, 0:1], rk[:, 1:2])
                    elif NA == 1:
                        nc.vector.tensor_copy(rsum[:, :], rk[:, 0:1])
                    else:
                        nc.vector.tensor_reduce(
                            rsum[:, :], rk[:, 0:NA],
                            axis=mybir.AxisListType.X, op=mybir.AluOpType.add,
                        )
                    rc = recip[:, h * NQT + qt:h * NQT + qt + 1]
                    nc.vector.reciprocal(rc, rsum[:, :])
                    a = aap.tile([128, s], f32, name="a", tag="a")
                    nc.vector.tensor_scalar_mul(a[:, :], u[:, :], rc)
                    nc.sync.dma_start(t["attn"][h, qs, :], a[:, :])

        # ---------------- output projection (partial) ----------------
        with (
            tc.tile_pool(name="fps", bufs=2, space="PSUM") as fps,
            tc.tile_pool(name="facc", bufs=2) as faccp,
            tc.tile_pool(name="ftmp", bufs=2) as ftmpp,
        ):
            for qt in range(NQT):
                qs = slice(qt * 128, qt * 128 + 128)
                acc = faccp.tile([128, D], f32, name="acc", tag="acc")
                rp = slice(0, DK) if (qt // 4) % 2 == 0 else slice(DK, 128)
                for h in range(HPC):
                    pf = fps.tile([128, D], f32, name="pf", tag="pf")
                    nc.tensor.matmul(pf[:, 0:512], ot_h[h][rp, qs],
                                     wo_h[h][rp, 0:512], start=True, stop=True)
                    nc.tensor.matmul(pf[:, 512:D], ot_h[h][rp, qs],
                                     wo_h[h][rp, 512:D], start=True, stop=True)
                    rc = recip[:, h * NQT + qt:h * NQT + qt + 1]
                    if h == 0:
                        nc.vector.tensor_scalar_mul(acc[:, :], pf[:, :], rc)
                    else:
                        tmp = ftmpp.tile([128, D], f32, name="tmp", tag="tmp")
                        nc.vector.tensor_scalar_mul(tmp[:, :], pf[:, :], rc)
                        nc.vector.tensor_add(acc[:, :], acc[:, :], tmp[:, :])
                nc.sync.dma_start(t["outp"][qs, :], acc[:, :])


def make_in_maps(inputs, s=S):
    """Shard + lay out the full inputs for the 8 cores."""
    q = np.asarray(inputs["query"], np.float32)
    k = np.asarray(inputs["key"], np.float32)
    v = np.asarray(inputs["value"], np.float32)
    Wq = np.asarray(inputs["Wq"], np.float32)
    Wk = np.asarray(inputs["Wk"], np.float32)
    Wv = np.asarray(inputs["Wv"], np.float32)
    Wo = np.asarray(inputs["Wo"], np.float32)
    bq = np.asarray(inputs["bq"], np.float32)
    bk = np.asarray(inputs["bk"], np.float32)
    bv = np.asarray(inputs["bv"], np.float32)

    def aug_x(x):  # [s, D] -> [D+1, s] bf16 with ones row
        out = np.empty((D + 1, s), BF16NP)
        out[:D] = np.ascontiguousarray(x.T).astype(BF16NP)
        out[D] = np.ones((s,), BF16NP)
        return out

    def aug_w(W, b, dims):  # -> [D+1, HD] bf16 with bias row
        out = np.empty((D + 1, HD), BF16NP)
        out[:D] = W[dims, :].T.astype(BF16NP)
        out[D] = b[dims].astype(BF16NP)
        return out

    in_maps = []
    for c in range(NCORES):
        b = c // 4
        g = c % 4
        dims = slice(g * HD, (g + 1) * HD)
        in_maps.append({
            "xqT": aug_x(q[b]),
            "xkT": aug_x(k[b]),
            "xvT": aug_x(v[b]),
            "wqT": aug_w(Wq, bq, dims),
            "wkT": aug_w(Wk, bk, dims),
            "wvT": aug_w(Wv, bv, dims),
            "woT": np.ascontiguousarray(Wo[:, dims].T).astype(BF16NP),
        })
    return in_maps


def _ensure_ntff_hook():
    """bass_utils' axon trace path imports antenv.axon_hooks, which is
    absent from this image; synthesize it around trn_agent_boot's ctypes
    NTFF driver so neuron-profile tracing works."""
    try:
        import antenv.axon_hooks  # noqa: F401
        return
    except ImportError:
        pass
    import types

    try:
        import antenv
    except ImportError:
        return
    state = {"hook": None, "built": False}

    def _get():
        if not state["built"]:
            state["built"] = True
            try:
                from trn_agent_boot.trn_boot import _ntff_profile_via_ctypes
                state["hook"] = _ntff_profile_via_ctypes("/opt/axon/libaxon_pjrt.so")
            except Exception:
                state["hook"] = None
        return state["hook"]

    def _set(h):
        state["hook"] = h
        state["built"] = True

    mod = types.ModuleType("antenv.axon_hooks")
    mod.get_axon_ntff_profile_hook = _get
    mod.set_axon_ntff_profile_hook = _set
    sys.modules["antenv.axon_hooks"] = mod
    antenv.axon_hooks = mod


def kernel(**inputs):
    from concourse import bass_utils

    _ensure_ntff_hook()
    if "nc" not in _CACHE:
        _CACHE["nc"] = _build(S)
    nc = _CACHE["nc"]

    in_maps = make_in_maps(inputs, S)
    res = bass_utils.run_bass_kernel_spmd(
        nc,
        in_maps,
        core_ids=list(range(NCORES)),
        trace=bool(os.environ.get("KERNEL_TRACE")),
    )
    _CACHE["last_result"] = res

    bo = np.asarray(inputs["bo"], np.float32)
    attn_full = np.empty((B, H, S, S), np.float32)
    out_full = np.zeros((B, S, D), np.float32)
    for c in range(NCORES):
        b = c // 4
        g = c % 4
        attn_full[b, g * HPC:(g + 1) * HPC] = res.results[c]["attn"]
        out_full[b] += res.results[c]["outp"]
    out_full += bo
    return out_full, attn_full
